# revision 1
# baseline (speedup 1.0000x reference)
"""MoE (top-2 of 8 experts) SwiGLU FFN on 8 Trainium2 NeuronCores.

Strategy (expert-parallel, per the sharding hint):
  - Router (x @ w_gate -> softmax -> top-2) computed host-side on jax-CPU with
    the exact ops the reference uses, so expert selection matches bit-for-bit.
    This is the "dispatch tokens by topk_idx" sharding step.
  - Core e receives only the tokens routed to expert e (gathered and
    transposed host-side to [C, cap]), plus expert e's weights. All cores run
    one SPMD program sized to cap = max tokens/expert (zero-padded).
  - Device computes y_e^T = wo_e^T @ (silu(wg_e^T x^T) * (wi_e^T x^T)) with
    bf16 matmuls accumulating in fp32 PSUM. Layout keeps tokens on the PSUM
    free dimension throughout, so no on-device transposes are needed:
    lhsT operands are the natural wi/wg [C,H] and wo [H,C] layouts.
  - Host combines: out[t] = val0[t]*y_{e0}[t] + val1[t]*y_{e1}[t].
"""

import numpy as np

import concourse.bass as bass
import concourse.mybir as mybir
import concourse.tile as tile
from concourse.bass_utils import run_bass_kernel_spmd

N_CORES = 8
N_EXPERTS = 8
TOP_K = 2
B, T, C, H = 4, 2048, 1024, 2048
CC = C // 128           # contraction chunks over C
HH = H // 128           # chunks over H
TOK_TILE = 512          # tokens per PSUM tile (one fp32 bank)
HBW = 512               # stage-1 weight block width (columns of H)
CBW = 512               # stage-2 weight block width (columns of C)


def _split_multi_waits(nc, max_waits=1):
    """This walrus build rejects >1 sync-wait per instruction. Peel extra
    waits onto single-wait EventSemaphore instructions inserted just before,
    on the same engine (identical blocking semantics)."""
    n_split = 0
    for fn in nc.m.functions:
        for bb in fn.blocks:
            out = []
            changed = False
            for inst in bb.instructions:
                si = inst.sync_info
                waits = list(si.on_wait) if si is not None else []
                if len(waits) > max_waits:
                    head, keep = waits[:-max_waits], waits[-max_waits:]
                    for j, w in enumerate(head):
                        out.append(mybir.InstEventSemaphore(
                            name=f"{inst.name}-wspl{j}",
                            engine=inst.engine,
                            sync_info=mybir.SyncInfo(on_wait=[w], on_update=[]),
                        ))
                    inst.sync_info = mybir.SyncInfo(
                        on_wait=keep, on_update=list(si.on_update))
                    changed = True
                    n_split += 1
                out.append(inst)
            if changed:
                bb.instructions = out
    return n_split


def build_program(cap, reps=1):
    """One SPMD program: expert FFN over [cap] tokens (token dim = PSUM free
    dim everywhere). reps>1 repeats the whole compute (for timing)."""
    assert cap % 128 == 0
    nc = bass.Bass()
    xt = nc.dram_tensor("xt", [C, cap], mybir.dt.float32, kind="ExternalInput")
    wi = nc.dram_tensor("wi", [C, H], mybir.dt.float32, kind="ExternalInput")
    wg = nc.dram_tensor("wg", [C, H], mybir.dt.float32, kind="ExternalInput")
    wo = nc.dram_tensor("wo", [H, C], mybir.dt.float32, kind="ExternalInput")
    yt = nc.dram_tensor("yt", [C, cap], mybir.dt.float32, kind="ExternalOutput")

    tok_tiles = [(t0, min(TOK_TILE, cap - t0)) for t0 in range(0, cap, TOK_TILE)]

    with tile.TileContext(nc) as tc:
        with tc.tile_pool(name="xb", bufs=1) as xb_pool, \
             tc.tile_pool(name="w1", bufs=2) as w1_pool, \
             tc.tile_pool(name="hT", bufs=1) as h_pool, \
             tc.tile_pool(name="w2", bufs=2) as w2_pool, \
             tc.tile_pool(name="sg", bufs=3) as sg_pool, \
             tc.tile_pool(name="yo", bufs=3) as yo_pool, \
             tc.tile_pool(name="ps", bufs=2, space="PSUM") as ps_pool, \
             tc.tile_pool(name="ps2", bufs=3, space="PSUM") as ps2_pool:

            for _rep in range(reps):
                # ---- load tokens, cast f32 -> bf16 during DMA (SWDGE) ----
                xb = xb_pool.tile([128, CC * cap], mybir.dt.bfloat16, tag="xb")
                for cc in range(CC):
                    nc.gpsimd.dma_start(xb[:, cc * cap:(cc + 1) * cap],
                                        xt[cc * 128:(cc + 1) * 128, :])

                # hT = silu(x@wg) * (x@wi), transposed: [H, cap] in bf16
                hT = h_pool.tile([128, HH * cap], mybir.dt.bfloat16, tag="hT")

                # ---- stage 1 ----
                for hb0 in range(0, H, HBW):
                    wib = w1_pool.tile([128, CC * HBW], mybir.dt.bfloat16,
                                       tag="wib")
                    wgb = w1_pool.tile([128, CC * HBW], mybir.dt.bfloat16,
                                       tag="wgb")
                    for cc in range(CC):
                        nc.gpsimd.dma_start(
                            wib[:, cc * HBW:(cc + 1) * HBW],
                            wi[cc * 128:(cc + 1) * 128, hb0:hb0 + HBW])
                        nc.gpsimd.dma_start(
                            wgb[:, cc * HBW:(cc + 1) * HBW],
                            wg[cc * 128:(cc + 1) * 128, hb0:hb0 + HBW])
                    for hi in range(HBW // 128):
                        hh = hb0 // 128 + hi
                        for t0, tw in tok_tiles:
                            ps_u = ps_pool.tile([128, TOK_TILE],
                                                mybir.dt.float32, tag="psu")
                            ps_g = ps_pool.tile([128, TOK_TILE],
                                                mybir.dt.float32, tag="psg")
                            for cc in range(CC):
                                nc.tensor.matmul(
                                    ps_u[:, :tw],
                                    wib[:, cc * HBW + hi * 128:
                                        cc * HBW + (hi + 1) * 128],
                                    xb[:, cc * cap + t0: cc * cap + t0 + tw],
                                    start=(cc == 0), stop=(cc == CC - 1))
                            for cc in range(CC):
                                nc.tensor.matmul(
                                    ps_g[:, :tw],
                                    wgb[:, cc * HBW + hi * 128:
                                        cc * HBW + (hi + 1) * 128],
                                    xb[:, cc * cap + t0: cc * cap + t0 + tw],
                                    start=(cc == 0), stop=(cc == CC - 1))
                            sg = sg_pool.tile([128, TOK_TILE],
                                              mybir.dt.float32, tag="sg")
                            nc.scalar.activation(
                                sg[:, :tw], ps_g[:, :tw],
                                mybir.ActivationFunctionType.Silu)
                            nc.vector.tensor_mul(
                                hT[:, hh * cap + t0: hh * cap + t0 + tw],
                                ps_u[:, :tw], sg[:, :tw])

                # ---- stage 2: yT = wo^T @ hT ----
                for cb0 in range(0, C, CBW):
                    wob = w2_pool.tile([128, HH * CBW], mybir.dt.bfloat16,
                                       tag="wob")
                    for hh in range(HH):
                        nc.gpsimd.dma_start(
                            wob[:, hh * CBW:(hh + 1) * CBW],
                            wo[hh * 128:(hh + 1) * 128, cb0:cb0 + CBW])
                    for ci in range(CBW // 128):
                        c0 = cb0 + ci * 128
                        for t0, tw in tok_tiles:
                            ps_y = ps2_pool.tile([128, TOK_TILE],
                                                 mybir.dt.float32, tag="psy")
                            for hh in range(HH):
                                nc.tensor.matmul(
                                    ps_y[:, :tw],
                                    wob[:, hh * CBW + ci * 128:
                                        hh * CBW + (ci + 1) * 128],
                                    hT[:, hh * cap + t0: hh * cap + t0 + tw],
                                    start=(hh == 0), stop=(hh == HH - 1))
                            yo = yo_pool.tile([128, TOK_TILE],
                                              mybir.dt.float32, tag="yo")
                            nc.vector.tensor_copy(yo[:, :tw], ps_y[:, :tw])
                            nc.sync.dma_start(yt[c0:c0 + 128, t0:t0 + tw],
                                              yo[:, :tw])
    _split_multi_waits(nc)
    return nc


def _route(x, w_gate):
    """Host-side router. Runs the exact reference ops on jax-CPU so the
    top-2 selection and gate values match the reference bit-for-bit."""
    import jax
    import jax.numpy as jnp
    cpu = jax.devices("cpu")[0]
    with jax.default_device(cpu):
        xj = jnp.asarray(np.asarray(x))
        wj = jnp.asarray(np.asarray(w_gate))
        logits = jnp.einsum("btc,ce->bte", xj, wj)
        gates = jax.nn.softmax(logits, axis=-1)
        topk_vals, topk_idx = jax.lax.top_k(gates, TOP_K)
    return (np.asarray(topk_vals).reshape(-1, TOP_K),
            np.asarray(topk_idx).reshape(-1, TOP_K))


def kernel(x, w_gate, wi, wg, wo):
    x = np.asarray(x)
    wi, wg, wo = np.asarray(wi), np.asarray(wg), np.asarray(wo)
    N = B * T
    x_flat = np.ascontiguousarray(x.reshape(N, C))

    topk_vals, topk_idx = _route(x, w_gate)

    # token lists per expert (sorted), positions of each (token, slot) pair
    idx_lists = []
    pos = np.empty((N, TOP_K), dtype=np.int64)
    for e in range(N_EXPERTS):
        sel = (topk_idx == e)                      # [N, K] bool
        toks = np.flatnonzero(sel.any(axis=1))     # sorted token ids
        idx_lists.append(toks)
        pos_of = np.full(N, -1, dtype=np.int64)
        pos_of[toks] = np.arange(len(toks))
        for k in range(TOP_K):
            m = sel[:, k]
            pos[m, k] = pos_of[m]

    max_cnt = max(len(t) for t in idx_lists)
    cap = max(128, -(-max_cnt // 128) * 128)

    # dispatch: gather + transpose tokens per expert
    xT = np.ascontiguousarray(x_flat.T)            # [C, N]
    in_maps = []
    for e in range(N_EXPERTS):
        toks = idx_lists[e]
        xt_e = np.zeros((C, cap), dtype=np.float32)
        xt_e[:, :len(toks)] = xT[:, toks]
        in_maps.append({
            "xt": xt_e,
            "wi": np.ascontiguousarray(wi[e]),
            "wg": np.ascontiguousarray(wg[e]),
            "wo": np.ascontiguousarray(wo[e]),
        })

    nc = build_program(cap)
    res = run_bass_kernel_spmd(nc, in_maps, core_ids=list(range(N_CORES)))

    # combine: out[t] = sum_k vals[t,k] * y_{idx[t,k]}[t]
    Y = np.empty((N_EXPERTS, cap, C), dtype=np.float32)   # token-major
    for e in range(N_EXPERTS):
        Y[e] = res.results[e]["yt"].T
    out = (topk_vals[:, 0:1] * Y[topk_idx[:, 0], pos[:, 0], :]
           + topk_vals[:, 1:2] * Y[topk_idx[:, 1], pos[:, 1], :])
    return out.reshape(B, T, C).astype(np.float32)
